# revision 12
# baseline (speedup 1.0000x reference)
"""Trainium2 Bass kernel for nn_DIVLoss (retrieval_knn).

Math collapse (validated to 8e-8 against the reference): each row of
nn_label_matrix holds exactly TOPK ones, so
    pred_nn[k] = (TOPK/B) * colsum(pred_base)[target[k]]
               = (TOPK/B) * qhat[target[k]] . sum_b fhat[b]
    pred_sel[k] = fhat[perm[k]] . qhat[target[perm[k]]],  perm = stable
                  argsort(target)
    loss = mean_k softplus(SCALE * (pred_nn[k] - pred_sel[k]))

Split: host handles data routing + the O(C*D)/O(B) sides (row norms,
fsum, query@fsum, gathers, softplus+mean); the 8 cores do the O(B*D)
work: the 4096 feature.query row dots.

Device strategy: after the stable sort by class, each 128-row tile's
classes span a narrow window (<=36 for this distribution; padded to
W=48).  So the row dots become 8 PE matmuls per tile:
    psum[m, w] += F_chunk^T[k, m] @ Qwin_chunk[k, w]   (k = d-chunk)
with F and the class-window queries shipped as fp8_e3m4 (1-byte, 4
mantissa bits; end-to-end rel err ~3e-4).  The per-row dot is then the
psum entry at the row's class offset; DVE extracts it with one fused
scalar_tensor_tensor against a host-built mask that carries
sz = 1/(|f||q|) at the hot column (so the multiply also normalizes),
and z = znn - d*sz closes per-sample logit difference on device.
Host computes loss = mean(softplus(SCALE*z)).

DMA plan (all HWDGE on the Sync sequencer, minimal instruction count
because descriptor-gen and the transfer bus are both serialized
devices): two fp8 transfers (tiles 0-1, tiles 2-3) of interleaved
[F^T | Qwin] blocks with 2.8KB contiguous lines, one small bf16
transfer with the masks + znn, one 2KB output.
"""

import numpy as np

N_CORES = 8
B = 4096
D = 1024
C = 1000
ROWS = B // N_CORES          # 512 rows per core
T = ROWS // 128              # 4 row-tiles of 128 partitions
CH = D // 128                # 8 contraction chunks
SCALE = 100.0
TOPK = 10.0

_cache = {}


def _build(W):
    import concourse.bacc as bacc
    import concourse.bass as bass
    import concourse.mybir as mybir
    import concourse.tile as tile

    f32 = mybir.dt.float32
    bf16 = mybir.dt.bfloat16
    f8 = mybir.dt.float8e3
    ALU = mybir.AluOpType

    nc = bacc.Bacc(
        "TRN2",
        target_bir_lowering=False,
        debug=False,
        enable_asserts=False,
        num_devices=N_CORES,
    )

    cq_d = nc.dram_tensor("cq", [128, T, CH, 128 + W], f8, kind="ExternalInput")
    mz_d = nc.dram_tensor("mz", [128, T, W + 1], bf16, kind="ExternalInput")
    zout_d = nc.dram_tensor("zout", [128, T], f32, kind="ExternalOutput")

    with tile.TileContext(nc) as tc:
        with (
            tc.tile_pool(name="sbuf", bufs=1) as pool,
            tc.tile_pool(name="psum", bufs=1, space=bass.MemorySpace.PSUM) as psum,
        ):
            cq = pool.tile([128, T, CH, 128 + W], f8, tag="cq")
            mz = pool.tile([128, T, W + 1], bf16, tag="mz")
            scratch = pool.tile([128, T, W], f32, tag="scratch")
            dsz = pool.tile([128, T], f32, tag="dsz")
            z = pool.tile([128, T], f32, tag="z")
            ps = [
                psum.tile([128, 512], f32, name=f"ps{t}", tag=f"ps{t}")
                for t in range(T)
            ]

            nc.sync.dma_start(cq[:, 0:2], cq_d[:, 0:2])
            nc.sync.dma_start(cq[:, 2:3], cq_d[:, 2:3])
            nc.sync.dma_start(cq[:, 3:4], cq_d[:, 3:4])
            nc.sync.dma_start(mz[:], mz_d[:])

            for t in range(T):
                for c in range(CH):
                    nc.tensor.matmul(
                        ps[t][:, 0:W],
                        cq[:, t, c, 0:128],
                        cq[:, t, c, 128 : 128 + W],
                        start=(c == 0),
                        stop=(c == CH - 1),
                    )
                # dsz[:, t] = sum(psum * (-sz onehot)) = -d*sz
                nc.vector.scalar_tensor_tensor(
                    scratch[:, t],
                    ps[t][:, 0:W],
                    1.0,
                    mz[:, t, 0:W],
                    ALU.mult,
                    ALU.mult,
                    accum_out=dsz[:, t : t + 1],
                )

            # z = znn + (-d*sz)  (znn rides in the last mz column)
            nc.vector.tensor_add(z[:], mz[:, :, W], dsz[:])
            nc.sync.dma_start(zout_d[:], z[:])

    nc.compile()
    return nc


def _host_prep(feature, query, target, W):
    import ml_dtypes

    e3 = ml_dtypes.float8_e3m4
    bf = ml_dtypes.bfloat16

    perm = np.argsort(target, kind="stable")
    tp = target[perm]                                   # sorted classes per row

    rf = 1.0 / np.sqrt((feature * feature).sum(axis=1))     # [B]
    rq = 1.0 / np.sqrt((query * query).sum(axis=1))         # [C]
    fsum = (feature * rf[:, None]).sum(axis=0, dtype=np.float32)
    u = (query @ fsum) * rq                                  # [C]

    sz = (rf[perm] * rq[tp]).astype(np.float32)              # [B] sel scale
    znn = ((TOPK / B) * u[target]).astype(np.float32)        # [B] nn logit

    F8 = feature[perm].astype(e3)                            # [B, D]
    Q8pad = np.zeros((C + W, D), dtype=e3)
    Q8pad[:C] = query.astype(e3)

    in_maps = []
    for k in range(N_CORES):
        sl = slice(k * ROWS, (k + 1) * ROWS)
        tpc = tp[sl].reshape(T, 128)
        clo = tpc[:, 0]
        # [128, T, CH, 128] stationary F^T blocks
        ftc = F8[sl].reshape(T, 128, CH, 128).transpose(3, 0, 2, 1)
        # [128, T, CH, W] moving class-window blocks
        qwc = np.stack(
            [
                Q8pad[clo[t] : clo[t] + W].reshape(W, CH, 128).transpose(2, 1, 0)
                for t in range(T)
            ],
            axis=1,
        )
        cqc = np.concatenate([ftc, qwc], axis=3)
        # masks: sz at the row's class offset, 0 elsewhere; znn last col
        mzc = np.zeros((128, T, W + 1), dtype=bf)
        off = tpc - clo[:, None]                              # [T, 128]
        mzc[np.arange(128)[None, :].repeat(T, 0).ravel(),
            np.arange(T)[:, None].repeat(128, 1).ravel(),
            off.ravel()] = (-sz[sl]).astype(bf)
        mzc[:, :, W] = znn[sl].reshape(T, 128).T.astype(bf)
        in_maps.append(
            {
                "cq": np.ascontiguousarray(cqc),
                "mz": np.ascontiguousarray(mzc),
            }
        )
    return in_maps


def kernel(feature, query, target):
    feature = np.ascontiguousarray(np.asarray(feature), dtype=np.float32)
    query = np.ascontiguousarray(np.asarray(query), dtype=np.float32)
    target = np.asarray(target)

    perm = np.argsort(target, kind="stable")
    tp = target[perm].reshape(B // 128, 128)
    maxwin = int((tp[:, -1] - tp[:, 0]).max()) + 1
    W = max(40, ((maxwin + 7) // 8) * 8)

    if ("nc", W) not in _cache:
        _cache[("nc", W)] = _build(W)
    nc = _cache[("nc", W)]

    in_maps = _host_prep(feature, query, target, W)

    from concourse.bass_utils import run_bass_kernel_spmd

    res = run_bass_kernel_spmd(
        nc,
        in_maps,
        core_ids=list(range(N_CORES)),
        trace=bool(getattr(kernel, "_trace", False)),
        tmpdir=getattr(kernel, "_tmpdir", None),
    )
    kernel.last_results = res

    z = np.concatenate(
        [r["zout"].astype(np.float64).T.reshape(ROWS) for r in res.results]
    )
    loss = np.mean(np.logaddexp(0.0, SCALE * z))
    return np.asarray(loss, dtype=np.float32)


# revision 13
# speedup vs baseline: 1.0752x; 1.0752x over previous
"""Trainium2 Bass kernel for nn_DIVLoss (retrieval_knn).

Math collapse (validated to 8e-8 against the reference): each row of
nn_label_matrix holds exactly TOPK ones, so
    pred_nn[k] = (TOPK/B) * colsum(pred_base)[target[k]]
               = (TOPK/B) * qhat[target[k]] . sum_b fhat[b]
    pred_sel[k] = fhat[perm[k]] . qhat[target[perm[k]]],  perm = stable
                  argsort(target)
    loss = mean_k softplus(SCALE * (pred_nn[k] - pred_sel[k]))

Split: host handles data routing + the O(C*D)/O(B) sides (row norms,
fsum, query@fsum, gathers, softplus+mean); the 8 cores do the O(B*D)
work: the 4096 feature.query row dots.

Device strategy: after the stable sort by class, each 128-row tile's
classes span a narrow window (<=36 for this distribution; padded to
W=48).  So the row dots become 8 PE matmuls per tile:
    psum[m, w] += F_chunk^T[k, m] @ Qwin_chunk[k, w]   (k = d-chunk)
with F and the class-window queries shipped as fp8_e3m4 (1-byte, 4
mantissa bits; end-to-end rel err ~3e-4).  The per-row dot is then the
psum entry at the row's class offset; DVE extracts it with one fused
scalar_tensor_tensor against a host-built mask that carries
sz = 1/(|f||q|) at the hot column (so the multiply also normalizes),
and z = znn - d*sz closes per-sample logit difference on device.
Host computes loss = mean(softplus(SCALE*z)).

DMA plan (all HWDGE on the Sync sequencer, minimal instruction count
because descriptor-gen and the transfer bus are both serialized
devices): two fp8 transfers (tiles 0-1, tiles 2-3) of interleaved
[F^T | Qwin] blocks with 2.8KB contiguous lines, one small bf16
transfer with the masks + znn, one 2KB output.
"""

import numpy as np

N_CORES = 8
B = 4096
D = 1024
C = 1000
ROWS = B // N_CORES          # 512 rows per core
T = ROWS // 128              # 4 row-tiles of 128 partitions
CH = D // 128                # 8 contraction chunks
SCALE = 100.0
TOPK = 10.0

_cache = {}


def _build(W):
    import concourse.bacc as bacc
    import concourse.bass as bass
    import concourse.mybir as mybir
    import concourse.tile as tile

    f32 = mybir.dt.float32
    bf16 = mybir.dt.bfloat16
    f8 = mybir.dt.float8e3
    ALU = mybir.AluOpType

    nc = bacc.Bacc(
        "TRN2",
        target_bir_lowering=False,
        debug=False,
        enable_asserts=False,
        num_devices=N_CORES,
    )

    cq_d = nc.dram_tensor("cq", [128, T, CH, 128 + W], f8, kind="ExternalInput")
    mz_d = nc.dram_tensor("mz", [128, T, W + 1], bf16, kind="ExternalInput")
    zout_d = nc.dram_tensor("zout", [128, T], f32, kind="ExternalOutput")

    with tile.TileContext(nc) as tc:
        with (
            tc.tile_pool(name="sbuf", bufs=1) as pool,
            tc.tile_pool(name="psum", bufs=1, space=bass.MemorySpace.PSUM) as psum,
        ):
            cq = pool.tile([128, T, CH, 128 + W], f8, tag="cq")
            mz = pool.tile([128, T, W + 1], bf16, tag="mz")
            scratch = pool.tile([128, T, W], f32, tag="scratch")
            dsz = pool.tile([128, T], f32, tag="dsz")
            z = pool.tile([128, T], f32, tag="z")
            ps = [
                psum.tile([128, 512], f32, name=f"ps{t}", tag=f"ps{t}")
                for t in range(T)
            ]

            nc.sync.dma_start(cq[:, 0:2], cq_d[:, 0:2])
            nc.sync.dma_start(cq[:, 2:4], cq_d[:, 2:4])
            nc.sync.dma_start(mz[:], mz_d[:])

            for t in range(T):
                for c in range(CH):
                    nc.tensor.matmul(
                        ps[t][:, 0:W],
                        cq[:, t, c, 0:128],
                        cq[:, t, c, 128 : 128 + W],
                        start=(c == 0),
                        stop=(c == CH - 1),
                    )
                # dsz[:, t] = sum(psum * (-sz onehot)) = -d*sz
                nc.vector.scalar_tensor_tensor(
                    scratch[:, t],
                    ps[t][:, 0:W],
                    1.0,
                    mz[:, t, 0:W],
                    ALU.mult,
                    ALU.mult,
                    accum_out=dsz[:, t : t + 1],
                )

            # z = znn + (-d*sz)  (znn rides in the last mz column)
            nc.vector.tensor_add(z[:], mz[:, :, W], dsz[:])
            nc.sync.dma_start(zout_d[:], z[:])

    nc.compile()
    return nc


def _host_prep(feature, query, target, W):
    import ml_dtypes

    e3 = ml_dtypes.float8_e3m4
    bf = ml_dtypes.bfloat16

    perm = np.argsort(target, kind="stable")
    tp = target[perm]                                   # sorted classes per row

    rf = 1.0 / np.sqrt((feature * feature).sum(axis=1))     # [B]
    rq = 1.0 / np.sqrt((query * query).sum(axis=1))         # [C]
    fsum = (feature * rf[:, None]).sum(axis=0, dtype=np.float32)
    u = (query @ fsum) * rq                                  # [C]

    sz = (rf[perm] * rq[tp]).astype(np.float32)              # [B] sel scale
    znn = ((TOPK / B) * u[target]).astype(np.float32)        # [B] nn logit

    F8 = feature[perm].astype(e3)                            # [B, D]
    Q8pad = np.zeros((C + W, D), dtype=e3)
    Q8pad[:C] = query.astype(e3)

    in_maps = []
    for k in range(N_CORES):
        sl = slice(k * ROWS, (k + 1) * ROWS)
        tpc = tp[sl].reshape(T, 128)
        clo = tpc[:, 0]
        # [128, T, CH, 128] stationary F^T blocks
        ftc = F8[sl].reshape(T, 128, CH, 128).transpose(3, 0, 2, 1)
        # [128, T, CH, W] moving class-window blocks
        qwc = np.stack(
            [
                Q8pad[clo[t] : clo[t] + W].reshape(W, CH, 128).transpose(2, 1, 0)
                for t in range(T)
            ],
            axis=1,
        )
        cqc = np.concatenate([ftc, qwc], axis=3)
        # masks: sz at the row's class offset, 0 elsewhere; znn last col
        mzc = np.zeros((128, T, W + 1), dtype=bf)
        off = tpc - clo[:, None]                              # [T, 128]
        mzc[np.arange(128)[None, :].repeat(T, 0).ravel(),
            np.arange(T)[:, None].repeat(128, 1).ravel(),
            off.ravel()] = (-sz[sl]).astype(bf)
        mzc[:, :, W] = znn[sl].reshape(T, 128).T.astype(bf)
        in_maps.append(
            {
                "cq": np.ascontiguousarray(cqc),
                "mz": np.ascontiguousarray(mzc),
            }
        )
    return in_maps


def kernel(feature, query, target):
    feature = np.ascontiguousarray(np.asarray(feature), dtype=np.float32)
    query = np.ascontiguousarray(np.asarray(query), dtype=np.float32)
    target = np.asarray(target)

    perm = np.argsort(target, kind="stable")
    tp = target[perm].reshape(B // 128, 128)
    maxwin = int((tp[:, -1] - tp[:, 0]).max()) + 1
    W = max(40, ((maxwin + 7) // 8) * 8)

    if ("nc", W) not in _cache:
        _cache[("nc", W)] = _build(W)
    nc = _cache[("nc", W)]

    in_maps = _host_prep(feature, query, target, W)

    from concourse.bass_utils import run_bass_kernel_spmd

    res = run_bass_kernel_spmd(
        nc,
        in_maps,
        core_ids=list(range(N_CORES)),
        trace=bool(getattr(kernel, "_trace", False)),
        tmpdir=getattr(kernel, "_tmpdir", None),
    )
    kernel.last_results = res

    z = np.concatenate(
        [r["zout"].astype(np.float64).T.reshape(ROWS) for r in res.results]
    )
    loss = np.mean(np.logaddexp(0.0, SCALE * z))
    return np.asarray(loss, dtype=np.float32)
